# revision 21
# baseline (speedup 1.0000x reference)
"""Trainium2 Bass kernel for nn_AlwGAT (GAT-style message passing), v2.

Math (equivalent to the reference):
  self = x[:, :36]; others = x[:, 36:].reshape(B, 19, 28)
  att  = softmax_j(others_j . Wa[36:])          # self-part cancels (shift inv.)
  out  = self @ A_self + (sum_j att_j * others_j) @ A_pool + c
where
  A_self = We[:36] @ Wo[:64] + (Ws[:36] + Ws[36:]) @ Wo[64:]
  A_pool = We[36:] @ Wo[:64]
  c      = be @ Wo[:64] + bs @ Wo[64:] + bo     (added on host; zeros here)

Dataflow (per 256-row tile, 2 half-blocks of 128 rows):
  host: x cast to bf16 and padded 568->640; row r0+16p+s lives on DRAM so
     that partition p gets rows 16p..16p+15 -> 20KB contiguous per-partition
     DMA runs (line rate), and the same (p s) mapping makes the output DMA
     contiguous too.
  ACT-queue HWDGE load: xbig [128, 16, 640] bf16 per 8-tile group
  XBAR DMA-transpose (sync queue, SBUF->SBUF): per 2 tiles,
     xt[p, m, r] = xbig[r, m//5, 128*(m%5)+p] -- feature chunks on
     partitions; junk pad partitions in chunk 4 are never read
  PE logits: lT[19, 2*128] = sum_c WL_c^T @ xt_c  (10 mm, h-outer so the
     two PSUM accumulation groups in one bank stay contiguous)
  ACT exp -> eT bf16; PE ones-matmul -> s per row; DVE recip -> rr
  PE erep: e broadcast to features via 0/1 selector, c-major 256-col mms
     (ones rows for self features make erep_self = s, folding the softmax
     denominator)
  DVE: sp = xt * erep (bf16)
  PE final: out[128, 2, 64] = sum_c sp_c^T @ FW_c   (data-stationary, ap=64)
  ACT: out_sbuf = out_psum * (1/s)  (Copy activation, per-partition scale)
  sync DMA out per 8-tile group.
"""

import os
import sys

if "/opt/trn_rl_repo" not in sys.path:
    sys.path.insert(0, "/opt/trn_rl_repo")

import numpy as np

SELF = 36
OTH = 28
J = 19
H = 64
OBS = SELF + OTH * J  # 568
NCORES = 8
BATCH = 65536
ROWS_PER_CORE = BATCH // NCORES  # 8192
TILE_ROWS = 256
NT = ROWS_PER_CORE // TILE_ROWS  # 32
F = [128, 128, 128, 128, 56]  # real features per chunk (5 x 128 covers 640)
NCH = 5
PADF = 640  # 5*128, XBAR needs free %128
GRP = 8  # tiles per load/store group
NG = NT // GRP

_CACHE = {}


def _build_nc():
    import concourse.bass as bass  # noqa: F401
    import concourse.tile as tile
    from concourse import bacc, mybir

    f32 = mybir.dt.float32
    bf16 = mybir.dt.bfloat16

    nc = bacc.Bacc("TRN2", debug=False)
    x_d = nc.dram_tensor(
        "x_in", [ROWS_PER_CORE, PADF], bf16, kind="ExternalInput"
    ).ap()
    wl_d = nc.dram_tensor("wl_in", [128, NCH, J + 1], f32, kind="ExternalInput").ap()
    b_d = nc.dram_tensor("bsel_in", [J, NCH, 128], f32, kind="ExternalInput").ap()
    fw_d = nc.dram_tensor("fw_in", [128, NCH, H], f32, kind="ExternalInput").ap()
    out_d = nc.dram_tensor("out", [ROWS_PER_CORE, H], f32, kind="ExternalOutput").ap()

    Exp = mybir.ActivationFunctionType.Exp
    Copy = mybir.ActivationFunctionType.Copy

    with tile.TileContext(nc) as tc:
        with (
            tc.tile_pool(name="consts", bufs=1) as consts,
            tc.tile_pool(name="xbig", bufs=2) as xbig_pool,
            tc.tile_pool(name="xt", bufs=3) as xt_pool,
            tc.tile_pool(name="eT", bufs=2) as eT_pool,
            tc.tile_pool(name="rr", bufs=2) as r_pool,
            tc.tile_pool(name="sp", bufs=2) as sp_pool,
            tc.tile_pool(name="obig", bufs=2) as obig_pool,
            tc.tile_pool(name="psLT", bufs=2, space="PSUM") as lt_pool,
            tc.tile_pool(name="psER", bufs=1, space="PSUM") as erep_pool,
            tc.tile_pool(name="psOUT", bufs=2, space="PSUM") as ops_pool,
        ):
            # constants: stage f32, convert once to bf16
            wl_st = consts.tile([128, NCH, J + 1], f32)
            nc.sync.dma_start(out=wl_st, in_=wl_d)
            wl_sb = consts.tile([128, NCH, J + 1], bf16)
            nc.scalar.copy(out=wl_sb, in_=wl_st)
            b_st = consts.tile([J, NCH, 128], f32)
            nc.sync.dma_start(out=b_st, in_=b_d)
            b_sb = consts.tile([J, NCH, 128], bf16)
            nc.scalar.copy(out=b_sb, in_=b_st)
            fw_st = consts.tile([128, NCH, H], f32)
            nc.sync.dma_start(out=fw_st, in_=fw_d)
            fw_sb = consts.tile([128, NCH, H], bf16)
            nc.scalar.copy(out=fw_sb, in_=fw_st)
            ones_st = consts.tile([J, 1], f32)
            nc.vector.memset(ones_st, 1.0)
            ones_bf = consts.tile([J, 1], bf16)
            nc.scalar.copy(out=ones_bf, in_=ones_st)

            st = {}

            def s_load(t):
                # x arrives bf16 640-padded from the host: row r0+16p+s lives
                # on partition p, slot s -> one 20 KB contiguous run per
                # partition (line-rate DMA, no on-device cast needed).
                if t % GRP:
                    return
                g = t // GRP
                r0 = g * GRP * TILE_ROWS
                xb = xbig_pool.tile([128, 2 * GRP, PADF], bf16, tag="xbig")
                nc.scalar.dma_start(
                    out=xb,
                    in_=x_d[r0 : r0 + GRP * TILE_ROWS, :].rearrange(
                        "(p s) f -> p s f", p=128
                    ),
                )
                st[("xb", g)] = xb
                ob = obig_pool.tile([128, 2 * GRP, H], f32, tag="obig")
                st[("ob", g)] = ob

            def s_xbar(t):
                if t % 2:
                    return
                xb = st[("xb", t // GRP)]
                hh = (t % GRP) * 2
                xt = xt_pool.tile([128, 4 * NCH, 128], bf16, tag="xt")
                nc.sync.dma_start(out=xt, in_=xb[:, hh : hh + 4, :], transpose=True)
                st[("xt", t // 2)] = xt

            def s_logits(t):
                xt = st[("xt", t // 2)]
                mo = (t % 2) * 2 * NCH
                lt = lt_pool.tile([128, 512], f32, tag="lt")
                # h-outer: keep each PSUM accumulation group contiguous
                for h in range(2):
                    for c in range(NCH):
                        fc = F[c]
                        nc.tensor.matmul(
                            lt[0:J, 128 * h : 128 * (h + 1)],
                            wl_sb[0:fc, c, 0:J],
                            xt[0:fc, mo + NCH * h + c, :],
                            start=(c == 0),
                            stop=(c == NCH - 1),
                        )
                st[("lt", t)] = lt

            def s_att(t):
                lt = st[("lt", t)]
                eT = eT_pool.tile([J, 256], bf16, tag="eT")
                nc.scalar.activation(out=eT, in_=lt[0:J, 0:256], func=Exp)
                for h in range(2):
                    nc.tensor.matmul(
                        lt[:, 384 + h : 385 + h],
                        eT[:, 128 * h : 128 * (h + 1)],
                        ones_bf,
                        start=True,
                        stop=True,
                    )
                rr = r_pool.tile([128, 2], f32, tag="rr")
                nc.vector.reciprocal(out=rr, in_=lt[:, 384:386])
                st[("eT", t)] = eT
                st[("rr", t)] = rr
                del st[("lt", t)]

            def s_erep(t):
                eT = st[("eT", t)]
                # c-major layout: one 256-col matmul per chunk (both halves),
                # halving the 128-col LDWEIGHTS count
                er = erep_pool.tile([128, NCH, 2, 128], f32, tag="er")
                for c in range(NCH):
                    nc.tensor.matmul(
                        er[:, c, :, :],
                        b_sb[:, c, :],
                        eT,
                        start=True,
                        stop=True,
                    )
                st[("er", t)] = er
                del st[("eT", t)]

            def s_sp(t):
                xt = st[("xt", t // 2)]
                mo = (t % 2) * 2 * NCH
                er = st[("er", t)]
                sp = sp_pool.tile([128, 2, NCH, 128], bf16, tag="sp")
                # DVE muls per half (gpsimd cannot read PSUM); er is c-major
                for h in range(2):
                    nc.vector.tensor_mul(
                        sp[:, h, :, :],
                        xt[:, mo + NCH * h : mo + NCH * (h + 1), :],
                        er[:, :, h, :],
                    )
                st[("sp", t)] = sp
                del st[("er", t)]

            def s_final(t):
                sp = st.pop(("sp", t))
                rr = st.pop(("rr", t))
                ob = st[("ob", t // GRP)]
                ops = ops_pool.tile([128, 2, H], f32, tag="ops")
                for h in range(2):
                    for c in range(NCH):
                        fc = F[c]
                        nc.tensor.matmul(
                            ops[:, h, :],
                            sp[0:fc, h, c, :],
                            fw_sb[0:fc, c, :],
                            start=(c == 0),
                            stop=(c == NCH - 1),
                        )
                    nc.scalar.activation(
                        out=ob[:, 2 * (t % GRP) + h, :],
                        in_=ops[:, h, :],
                        func=Copy,
                        scale=rr[:, h : h + 1],
                    )
                if t % GRP == GRP - 1:
                    g = t // GRP
                    r0 = g * GRP * TILE_ROWS
                    nc.sync.dma_start(
                        out=out_d[r0 : r0 + GRP * TILE_ROWS, :].rearrange(
                            "(p s) f -> p s f", p=128
                        ),
                        in_=st.pop(("ob", g)),
                    )
                    st.pop(("xb", g), None)

            stages = [
                (s_load, 0),
                (s_xbar, 0),
                (s_logits, 2),
                (s_att, 3),
                (s_erep, 3),
                (s_sp, 4),
                (s_final, 5),
            ]
            for r in range(NT + 5):
                for fn, off in stages:
                    tt = r - off
                    if 0 <= tt < NT:
                        fn(tt)

    nc.compile()
    return nc


def _fold_weights(Wa, ba, We, be, Ws, bs, Wo, bo):
    Wa = np.asarray(Wa, np.float64)
    We = np.asarray(We, np.float64)
    Ws = np.asarray(Ws, np.float64)
    Wo = np.asarray(Wo, np.float64)
    wa2 = Wa[SELF:, 0]  # [28]
    A_self = We[:SELF] @ Wo[:H] + (Ws[:SELF] + Ws[SELF:]) @ Wo[H:]  # [36, 64]
    A_pool = We[SELF:] @ Wo[:H]  # [28, 64]
    c = (
        np.asarray(be, np.float64) @ Wo[:H]
        + np.asarray(bs, np.float64) @ Wo[H:]
        + np.asarray(bo, np.float64)
    )  # [64]

    WLp = np.zeros((128, NCH, J + 1), np.float32)
    Bp = np.zeros((J, NCH, 128), np.float32)
    FWp = np.zeros((128, NCH, H), np.float32)
    for ch in range(NCH):
        for p in range(128):
            f = 128 * ch + p
            if f >= OBS:
                continue
            if f < SELF:
                Bp[:, ch, p] = 1.0  # ones block -> s for self features
                FWp[p, ch, :] = A_self[f]
            else:
                j0, k = divmod(f - SELF, OTH)
                WLp[p, ch, j0] = wa2[k]
                Bp[j0, ch, p] = 1.0
                FWp[p, ch, :] = A_pool[k]
    return WLp, Bp, FWp, c.astype(np.float32)


def kernel(x, Wa, ba, We, be, Ws, bs, Wo, bo):
    import ml_dtypes

    from concourse import bass_utils

    x = np.asarray(x, np.float32)
    assert x.shape == (BATCH, OBS), x.shape
    # host-side bf16 cast + pad to 640 (XBAR alignment): device loads
    # contiguous bf16 directly, no on-chip conversion
    xpad = np.zeros((BATCH, PADF), dtype=ml_dtypes.bfloat16)
    xpad[:, :OBS] = x.astype(ml_dtypes.bfloat16)

    WLp, Bp, FWp, c = _fold_weights(Wa, ba, We, be, Ws, bs, Wo, bo)

    if "nc" not in _CACHE:
        _CACHE["nc"] = _build_nc()
    nc = _CACHE["nc"]

    in_maps = []
    for i in range(NCORES):
        in_maps.append(
            {
                "x_in": xpad[i * ROWS_PER_CORE : (i + 1) * ROWS_PER_CORE],
                "wl_in": WLp,
                "bsel_in": Bp,
                "fw_in": FWp,
            }
        )

    res = bass_utils.run_bass_kernel_spmd(
        nc,
        in_maps,
        core_ids=list(range(NCORES)),
        trace=_CACHE.get("trace", False),
        **_CACHE.get("run_kwargs", {}),
    )
    _CACHE["last_results"] = res

    out = np.concatenate([np.asarray(res.results[i]["out"]) for i in range(NCORES)], 0)
    if np.any(c):
        out = out + c[None, :]
    return out.astype(np.float32)


# revision 23
# speedup vs baseline: 10167.8813x; 10167.8813x over previous
"""Trainium2 Bass kernel for nn_AlwGAT (GAT-style message passing), v2.

Math (equivalent to the reference):
  self = x[:, :36]; others = x[:, 36:].reshape(B, 19, 28)
  att  = softmax_j(others_j . Wa[36:])          # self-part cancels (shift inv.)
  out  = self @ A_self + (sum_j att_j * others_j) @ A_pool + c
where
  A_self = We[:36] @ Wo[:64] + (Ws[:36] + Ws[36:]) @ Wo[64:]
  A_pool = We[36:] @ Wo[:64]
  c      = be @ Wo[:64] + bs @ Wo[64:] + bo     (added on host; zeros here)

Dataflow (per 256-row tile, 2 half-blocks of 128 rows):
  host: x cast to bf16 and padded 568->640; row r0+16p+s lives on DRAM so
     that partition p gets rows 16p..16p+15 -> 20KB contiguous per-partition
     DMA runs (line rate), and the same (p s) mapping makes the output DMA
     contiguous too.
  ACT-queue HWDGE load: xbig [128, 16, 640] bf16 per 8-tile group
  XBAR DMA-transpose (sync queue, SBUF->SBUF): per 2 tiles,
     xt[p, m, r] = xbig[r, m//5, 128*(m%5)+p] -- feature chunks on
     partitions; junk pad partitions in chunk 4 are never read
  PE logits: lT[19, 2*128] = sum_c WL_c^T @ xt_c  (10 mm, h-outer so the
     two PSUM accumulation groups in one bank stay contiguous)
  ACT exp -> eT bf16; PE ones-matmul -> s per row; DVE recip -> rr
  PE erep: e broadcast to features via 0/1 selector, c-major 256-col mms
     (ones rows for self features make erep_self = s, folding the softmax
     denominator)
  DVE: sp = xt * erep (bf16)
  PE final: out[128, 2, 64] = sum_c sp_c^T @ FW_c   (data-stationary, ap=64)
  ACT: out_sbuf = out_psum * (1/s)  (Copy activation, per-partition scale)
  sync DMA out per 8-tile group.
"""

import os
import sys

if "/opt/trn_rl_repo" not in sys.path:
    sys.path.insert(0, "/opt/trn_rl_repo")

import numpy as np

SELF = 36
OTH = 28
J = 19
H = 64
OBS = SELF + OTH * J  # 568
NCORES = 8
BATCH = 65536
ROWS_PER_CORE = BATCH // NCORES  # 8192
TILE_ROWS = 256
NT = ROWS_PER_CORE // TILE_ROWS  # 32
F = [128, 128, 128, 128, 56]  # real features per chunk (5 x 128 covers 640)
NCH = 5
PADF = 640  # 5*128, XBAR needs free %128
GRP = 8  # tiles per load/store group
NG = NT // GRP

_CACHE = {}


def _build_nc():
    import concourse.bass as bass  # noqa: F401
    import concourse.tile as tile
    from concourse import bacc, mybir

    f32 = mybir.dt.float32
    bf16 = mybir.dt.bfloat16

    nc = bacc.Bacc("TRN2", debug=False)
    x_d = nc.dram_tensor(
        "x_in", [ROWS_PER_CORE, PADF], bf16, kind="ExternalInput"
    ).ap()
    wl_d = nc.dram_tensor("wl_in", [128, NCH, J + 1], f32, kind="ExternalInput").ap()
    b_d = nc.dram_tensor("bsel_in", [J, NCH, 128], f32, kind="ExternalInput").ap()
    fw_d = nc.dram_tensor("fw_in", [128, NCH, H], f32, kind="ExternalInput").ap()
    out_d = nc.dram_tensor("out", [ROWS_PER_CORE, H], f32, kind="ExternalOutput").ap()

    Exp = mybir.ActivationFunctionType.Exp
    Copy = mybir.ActivationFunctionType.Copy

    with tile.TileContext(nc) as tc:
        with (
            tc.tile_pool(name="consts", bufs=1) as consts,
            tc.tile_pool(name="xbig", bufs=2) as xbig_pool,
            tc.tile_pool(name="xt", bufs=3) as xt_pool,
            tc.tile_pool(name="eT", bufs=2) as eT_pool,
            tc.tile_pool(name="rr", bufs=2) as r_pool,
            tc.tile_pool(name="sp", bufs=2) as sp_pool,
            tc.tile_pool(name="obig", bufs=2) as obig_pool,
            tc.tile_pool(name="psLT", bufs=2, space="PSUM") as lt_pool,
            tc.tile_pool(name="psER", bufs=1, space="PSUM") as erep_pool,
            tc.tile_pool(name="psOUT", bufs=2, space="PSUM") as ops_pool,
        ):
            # constants: stage f32, convert once to bf16
            wl_st = consts.tile([128, NCH, J + 1], f32)
            nc.sync.dma_start(out=wl_st, in_=wl_d)
            wl_sb = consts.tile([128, NCH, J + 1], bf16)
            nc.scalar.copy(out=wl_sb, in_=wl_st)
            b_st = consts.tile([J, NCH, 128], f32)
            nc.sync.dma_start(out=b_st, in_=b_d)
            b_sb = consts.tile([J, NCH, 128], bf16)
            nc.scalar.copy(out=b_sb, in_=b_st)
            fw_st = consts.tile([128, NCH, H], f32)
            nc.sync.dma_start(out=fw_st, in_=fw_d)
            fw_sb = consts.tile([128, NCH, H], bf16)
            nc.scalar.copy(out=fw_sb, in_=fw_st)
            ones_st = consts.tile([J, 1], f32)
            nc.vector.memset(ones_st, 1.0)
            ones_bf = consts.tile([J, 1], bf16)
            nc.scalar.copy(out=ones_bf, in_=ones_st)

            st = {}

            def do_load(g):
                # x arrives bf16 640-padded from the host: row r0+16p+s lives
                # on partition p, slot s -> one 20 KB contiguous run per
                # partition (line-rate DMA, no on-device cast needed).  The
                # load goes on the gpsimd SWDGE queue so it cannot serialize
                # against the XBAR transposes on the sync HWDGE ring.
                r0 = g * GRP * TILE_ROWS
                xb = xbig_pool.tile([128, 2 * GRP, PADF], bf16, tag="xbig")
                nc.gpsimd.dma_start(
                    out=xb,
                    in_=x_d[r0 : r0 + GRP * TILE_ROWS, :].rearrange(
                        "(p s) f -> p s f", p=128
                    ),
                )
                st[("xb", g)] = xb
                ob = obig_pool.tile([128, 2 * GRP, H], f32, tag="obig")
                st[("ob", g)] = ob

            def s_load(t):
                # prefetch one group ahead (groups 0 and 1 primed pre-loop)
                if t % GRP:
                    return
                g = t // GRP + 2
                if g < NG:
                    do_load(g)

            def s_xbar(t):
                if t % 2:
                    return
                xb = st[("xb", t // GRP)]
                hh = (t % GRP) * 2
                xt = xt_pool.tile([128, 4 * NCH, 128], bf16, tag="xt")
                nc.sync.dma_start(out=xt, in_=xb[:, hh : hh + 4, :], transpose=True)
                st[("xt", t // 2)] = xt

            def s_logits(t):
                xt = st[("xt", t // 2)]
                mo = (t % 2) * 2 * NCH
                lt = lt_pool.tile([128, 512], f32, tag="lt")
                # h-outer: keep each PSUM accumulation group contiguous
                for h in range(2):
                    for c in range(NCH):
                        fc = F[c]
                        nc.tensor.matmul(
                            lt[0:J, 128 * h : 128 * (h + 1)],
                            wl_sb[0:fc, c, 0:J],
                            xt[0:fc, mo + NCH * h + c, :],
                            start=(c == 0),
                            stop=(c == NCH - 1),
                        )
                st[("lt", t)] = lt

            def s_att(t):
                lt = st[("lt", t)]
                eT = eT_pool.tile([J, 256], bf16, tag="eT")
                nc.scalar.activation(out=eT, in_=lt[0:J, 0:256], func=Exp)
                for h in range(2):
                    nc.tensor.matmul(
                        lt[:, 384 + h : 385 + h],
                        eT[:, 128 * h : 128 * (h + 1)],
                        ones_bf,
                        start=True,
                        stop=True,
                    )
                rr = r_pool.tile([128, 2], f32, tag="rr")
                nc.vector.reciprocal(out=rr, in_=lt[:, 384:386])
                st[("eT", t)] = eT
                st[("rr", t)] = rr
                del st[("lt", t)]

            def s_erep(t):
                eT = st[("eT", t)]
                # c-major layout: one 256-col matmul per chunk (both halves),
                # halving the 128-col LDWEIGHTS count
                er = erep_pool.tile([128, NCH, 2, 128], f32, tag="er")
                for c in range(NCH):
                    nc.tensor.matmul(
                        er[:, c, :, :],
                        b_sb[:, c, :],
                        eT,
                        start=True,
                        stop=True,
                    )
                st[("er", t)] = er
                del st[("eT", t)]

            def s_sp(t):
                xt = st[("xt", t // 2)]
                mo = (t % 2) * 2 * NCH
                er = st[("er", t)]
                sp = sp_pool.tile([128, 2, NCH, 128], bf16, tag="sp")
                # DVE muls per half (gpsimd cannot read PSUM); er is c-major
                for h in range(2):
                    nc.vector.tensor_mul(
                        sp[:, h, :, :],
                        xt[:, mo + NCH * h : mo + NCH * (h + 1), :],
                        er[:, :, h, :],
                    )
                st[("sp", t)] = sp
                del st[("er", t)]

            def s_final(t):
                sp = st.pop(("sp", t))
                rr = st.pop(("rr", t))
                ob = st[("ob", t // GRP)]
                ops = ops_pool.tile([128, 2, H], f32, tag="ops")
                for h in range(2):
                    for c in range(NCH):
                        fc = F[c]
                        nc.tensor.matmul(
                            ops[:, h, :],
                            sp[0:fc, h, c, :],
                            fw_sb[0:fc, c, :],
                            start=(c == 0),
                            stop=(c == NCH - 1),
                        )
                    nc.scalar.activation(
                        out=ob[:, 2 * (t % GRP) + h, :],
                        in_=ops[:, h, :],
                        func=Copy,
                        scale=rr[:, h : h + 1],
                    )
                if t % GRP == GRP - 1:
                    g = t // GRP
                    r0 = g * GRP * TILE_ROWS
                    nc.sync.dma_start(
                        out=out_d[r0 : r0 + GRP * TILE_ROWS, :].rearrange(
                            "(p s) f -> p s f", p=128
                        ),
                        in_=st.pop(("ob", g)),
                    )
                    st.pop(("xb", g), None)

            do_load(0)
            do_load(1)
            stages = [
                (s_load, 0),
                (s_xbar, 0),
                (s_logits, 2),
                (s_att, 3),
                (s_erep, 3),
                (s_sp, 4),
                (s_final, 5),
            ]
            for r in range(NT + 5):
                for fn, off in stages:
                    tt = r - off
                    if 0 <= tt < NT:
                        fn(tt)

    nc.compile()
    return nc


def _fold_weights(Wa, ba, We, be, Ws, bs, Wo, bo):
    Wa = np.asarray(Wa, np.float64)
    We = np.asarray(We, np.float64)
    Ws = np.asarray(Ws, np.float64)
    Wo = np.asarray(Wo, np.float64)
    wa2 = Wa[SELF:, 0]  # [28]
    A_self = We[:SELF] @ Wo[:H] + (Ws[:SELF] + Ws[SELF:]) @ Wo[H:]  # [36, 64]
    A_pool = We[SELF:] @ Wo[:H]  # [28, 64]
    c = (
        np.asarray(be, np.float64) @ Wo[:H]
        + np.asarray(bs, np.float64) @ Wo[H:]
        + np.asarray(bo, np.float64)
    )  # [64]

    WLp = np.zeros((128, NCH, J + 1), np.float32)
    Bp = np.zeros((J, NCH, 128), np.float32)
    FWp = np.zeros((128, NCH, H), np.float32)
    for ch in range(NCH):
        for p in range(128):
            f = 128 * ch + p
            if f >= OBS:
                continue
            if f < SELF:
                Bp[:, ch, p] = 1.0  # ones block -> s for self features
                FWp[p, ch, :] = A_self[f]
            else:
                j0, k = divmod(f - SELF, OTH)
                WLp[p, ch, j0] = wa2[k]
                Bp[j0, ch, p] = 1.0
                FWp[p, ch, :] = A_pool[k]
    return WLp, Bp, FWp, c.astype(np.float32)


def kernel(x, Wa, ba, We, be, Ws, bs, Wo, bo):
    import ml_dtypes

    from concourse import bass_utils

    x = np.asarray(x, np.float32)
    assert x.shape == (BATCH, OBS), x.shape
    # host-side bf16 cast + pad to 640 (XBAR alignment): device loads
    # contiguous bf16 directly, no on-chip conversion
    xpad = np.zeros((BATCH, PADF), dtype=ml_dtypes.bfloat16)
    xpad[:, :OBS] = x.astype(ml_dtypes.bfloat16)

    WLp, Bp, FWp, c = _fold_weights(Wa, ba, We, be, Ws, bs, Wo, bo)

    if "nc" not in _CACHE:
        _CACHE["nc"] = _build_nc()
    nc = _CACHE["nc"]

    in_maps = []
    for i in range(NCORES):
        in_maps.append(
            {
                "x_in": xpad[i * ROWS_PER_CORE : (i + 1) * ROWS_PER_CORE],
                "wl_in": WLp,
                "bsel_in": Bp,
                "fw_in": FWp,
            }
        )

    res = bass_utils.run_bass_kernel_spmd(
        nc,
        in_maps,
        core_ids=list(range(NCORES)),
        trace=_CACHE.get("trace", False),
        **_CACHE.get("run_kwargs", {}),
    )
    _CACHE["last_results"] = res

    out = np.concatenate([np.asarray(res.results[i]["out"]) for i in range(NCORES)], 0)
    if np.any(c):
        out = out + c[None, :]
    return out.astype(np.float32)


# revision 24
# speedup vs baseline: 10291.0285x; 1.0121x over previous
"""Trainium2 Bass kernel for nn_AlwGAT (GAT-style message passing), v2.

Math (equivalent to the reference):
  self = x[:, :36]; others = x[:, 36:].reshape(B, 19, 28)
  att  = softmax_j(others_j . Wa[36:])          # self-part cancels (shift inv.)
  out  = self @ A_self + (sum_j att_j * others_j) @ A_pool + c
where
  A_self = We[:36] @ Wo[:64] + (Ws[:36] + Ws[36:]) @ Wo[64:]
  A_pool = We[36:] @ Wo[:64]
  c      = be @ Wo[:64] + bs @ Wo[64:] + bo     (added on host; zeros here)

Dataflow (per 256-row tile, 2 half-blocks of 128 rows):
  host: x cast to bf16 and padded 568->640; row r0+16p+s lives on DRAM so
     that partition p gets rows 16p..16p+15 -> 20KB contiguous per-partition
     DMA runs (line rate), and the same (p s) mapping makes the output DMA
     contiguous too.
  ACT-queue HWDGE load: xbig [128, 16, 640] bf16 per 8-tile group
  XBAR DMA-transpose (sync queue, SBUF->SBUF): per 2 tiles,
     xt[p, m, r] = xbig[r, m//5, 128*(m%5)+p] -- feature chunks on
     partitions; junk pad partitions in chunk 4 are never read
  PE logits: lT[19, 2*128] = sum_c WL_c^T @ xt_c  (10 mm, h-outer so the
     two PSUM accumulation groups in one bank stay contiguous)
  ACT exp -> eT bf16; PE ones-matmul -> s per row; DVE recip -> rr
  PE erep: e broadcast to features via 0/1 selector, c-major 256-col mms
     (ones rows for self features make erep_self = s, folding the softmax
     denominator)
  DVE: sp = xt * erep (bf16)
  PE final: out[128, 2, 64] = sum_c sp_c^T @ FW_c   (data-stationary, ap=64)
  ACT: out_sbuf = out_psum * (1/s)  (Copy activation, per-partition scale)
  sync DMA out per 8-tile group.
"""

import os
import sys

if "/opt/trn_rl_repo" not in sys.path:
    sys.path.insert(0, "/opt/trn_rl_repo")

import numpy as np

SELF = 36
OTH = 28
J = 19
H = 64
OBS = SELF + OTH * J  # 568
NCORES = 8
BATCH = 65536
ROWS_PER_CORE = BATCH // NCORES  # 8192
TILE_ROWS = 256
NT = ROWS_PER_CORE // TILE_ROWS  # 32
F = [128, 128, 128, 128, 56]  # real features per chunk (5 x 128 covers 640)
FFIN = [128, 128, 128, 128, 57]  # final contraction includes the s-row (p=56,c=4)
H2 = H + 2  # final out cols: 64 outputs + s + pad
NCH = 5
PADF = 640  # 5*128, XBAR needs free %128
GRP = 8  # tiles per load/store group
NG = NT // GRP

_CACHE = {}


def _build_nc():
    import concourse.bass as bass  # noqa: F401
    import concourse.tile as tile
    from concourse import bacc, mybir

    f32 = mybir.dt.float32
    bf16 = mybir.dt.bfloat16

    nc = bacc.Bacc("TRN2", debug=False)
    x_d = nc.dram_tensor(
        "x_in", [ROWS_PER_CORE, PADF], bf16, kind="ExternalInput"
    ).ap()
    wl_d = nc.dram_tensor("wl_in", [128, NCH, J + 1], f32, kind="ExternalInput").ap()
    b_d = nc.dram_tensor("bsel_in", [J, NCH, 128], f32, kind="ExternalInput").ap()
    fw_d = nc.dram_tensor("fw_in", [128, NCH, H2], f32, kind="ExternalInput").ap()
    out_d = nc.dram_tensor("out", [ROWS_PER_CORE, H], f32, kind="ExternalOutput").ap()

    Exp = mybir.ActivationFunctionType.Exp
    Copy = mybir.ActivationFunctionType.Copy

    with tile.TileContext(nc) as tc:
        with (
            tc.tile_pool(name="consts", bufs=1) as consts,
            tc.tile_pool(name="xbig", bufs=2) as xbig_pool,
            tc.tile_pool(name="xt", bufs=4) as xt_pool,
            tc.tile_pool(name="eT", bufs=2) as eT_pool,
            tc.tile_pool(name="rr", bufs=2) as r_pool,
            tc.tile_pool(name="sp", bufs=2) as sp_pool,
            tc.tile_pool(name="obig", bufs=2) as obig_pool,
            tc.tile_pool(name="psLT", bufs=2, space="PSUM") as lt_pool,
            tc.tile_pool(name="psER", bufs=1, space="PSUM") as erep_pool,
            tc.tile_pool(name="psOUT", bufs=2, space="PSUM") as ops_pool,
        ):
            # constants: stage f32, convert once to bf16
            wl_st = consts.tile([128, NCH, J + 1], f32)
            nc.sync.dma_start(out=wl_st, in_=wl_d)
            wl_sb = consts.tile([128, NCH, J + 1], bf16)
            nc.scalar.copy(out=wl_sb, in_=wl_st)
            b_st = consts.tile([J, NCH, 128], f32)
            nc.sync.dma_start(out=b_st, in_=b_d)
            b_sb = consts.tile([J, NCH, 128], bf16)
            nc.scalar.copy(out=b_sb, in_=b_st)
            fw_st = consts.tile([128, NCH, H2], f32)
            nc.sync.dma_start(out=fw_st, in_=fw_d)
            fw_sb = consts.tile([128, NCH, H2], bf16)
            nc.scalar.copy(out=fw_sb, in_=fw_st)

            st = {}

            def do_load(g):
                # x arrives bf16 640-padded from the host: row r0+16p+s lives
                # on partition p, slot s -> one 20 KB contiguous run per
                # partition (line-rate DMA, no on-device cast needed).  The
                # load goes on the gpsimd SWDGE queue so it cannot serialize
                # against the XBAR transposes on the sync HWDGE ring.
                r0 = g * GRP * TILE_ROWS
                xb = xbig_pool.tile([128, 2 * GRP, PADF], bf16, tag="xbig")
                nc.scalar.dma_start(
                    out=xb,
                    in_=x_d[r0 : r0 + GRP * TILE_ROWS, :].rearrange(
                        "(p s) f -> p s f", p=128
                    ),
                )
                st[("xb", g)] = xb
                ob = obig_pool.tile([128, 2 * GRP, H], f32, tag="obig")
                st[("ob", g)] = ob

            def s_load(t):
                # prefetch one group ahead (groups 0 and 1 primed pre-loop)
                if t % GRP:
                    return
                g = t // GRP + 2
                if g < NG:
                    do_load(g)

            def s_xbar(t):
                if t % 2:
                    return
                xb = st[("xb", t // GRP)]
                hh = (t % GRP) * 2
                xt = xt_pool.tile([128, 4 * NCH, 128], bf16, tag="xt")
                nc.sync.dma_start(out=xt, in_=xb[:, hh : hh + 4, :], transpose=True)
                st[("xt", t // 2)] = xt

            def s_logits(t):
                xt = st[("xt", t // 2)]
                mo = (t % 2) * 2 * NCH
                lt = lt_pool.tile([128, 512], f32, tag="lt")
                # h-outer: keep each PSUM accumulation group contiguous
                for h in range(2):
                    for c in range(NCH):
                        fc = F[c]
                        nc.tensor.matmul(
                            lt[0:J, 128 * h : 128 * (h + 1)],
                            wl_sb[0:fc, c, 0:J],
                            xt[0:fc, mo + NCH * h + c, :],
                            start=(c == 0),
                            stop=(c == NCH - 1),
                        )
                st[("lt", t)] = lt

            def s_att(t):
                lt = st[("lt", t)]
                eT = eT_pool.tile([J, 256], bf16, tag="eT")
                nc.scalar.activation(out=eT, in_=lt[0:J, 0:256], func=Exp)
                st[("eT", t)] = eT
                del st[("lt", t)]

            def s_erep(t):
                eT = st[("eT", t)]
                # c-major layout: one 256-col matmul per chunk (both halves),
                # halving the 128-col LDWEIGHTS count
                er = erep_pool.tile([128, NCH, 2, 128], f32, tag="er")
                for c in range(NCH):
                    nc.tensor.matmul(
                        er[:, c, :, :],
                        b_sb[:, c, :],
                        eT,
                        start=True,
                        stop=True,
                    )
                st[("er", t)] = er
                del st[("eT", t)]

            def s_sp(t):
                xt = st[("xt", t // 2)]
                mo = (t % 2) * 2 * NCH
                er = st[("er", t)]
                sp = sp_pool.tile([128, 2, NCH, 128], bf16, tag="sp")
                # DVE muls per half (gpsimd cannot read PSUM); er is c-major
                for h in range(2):
                    nc.vector.tensor_mul(
                        sp[:, h, :, :],
                        xt[:, mo + NCH * h : mo + NCH * (h + 1), :],
                        er[:, :, h, :],
                    )
                st[("sp", t)] = sp
                del st[("er", t)]

            def s_final(t):
                sp = st.pop(("sp", t))
                ob = st[("ob", t // GRP)]
                ops = ops_pool.tile([128, 2, H2], f32, tag="ops")
                rr = r_pool.tile([128, 2], f32, tag="rr")
                for h in range(2):
                    for c in range(NCH):
                        fc = FFIN[c]
                        nc.tensor.matmul(
                            ops[:, h, :],
                            sp[0:fc, h, c, :],
                            fw_sb[0:fc, c, :],
                            start=(c == 0),
                            stop=(c == NCH - 1),
                        )
                    # col 64 of the matmul output is s = sum_j e_j (the
                    # pad-column/selector trick); normalize via its recip
                    nc.vector.reciprocal(
                        out=rr[:, h : h + 1], in_=ops[:, h, H : H + 1]
                    )
                    nc.scalar.activation(
                        out=ob[:, 2 * (t % GRP) + h, :],
                        in_=ops[:, h, 0:H],
                        func=Copy,
                        scale=rr[:, h : h + 1],
                    )
                if t % GRP == GRP - 1:
                    g = t // GRP
                    r0 = g * GRP * TILE_ROWS
                    nc.sync.dma_start(
                        out=out_d[r0 : r0 + GRP * TILE_ROWS, :].rearrange(
                            "(p s) f -> p s f", p=128
                        ),
                        in_=st.pop(("ob", g)),
                    )
                    st.pop(("xb", g), None)

            do_load(0)
            do_load(1)
            stages = [
                (s_load, 0),
                (s_xbar, 0),
                (s_logits, 2),
                (s_att, 3),
                (s_erep, 3),
                (s_sp, 4),
                (s_final, 5),
            ]
            for r in range(NT + 5):
                for fn, off in stages:
                    tt = r - off
                    if 0 <= tt < NT:
                        fn(tt)

    nc.compile()
    return nc


def _fold_weights(Wa, ba, We, be, Ws, bs, Wo, bo):
    Wa = np.asarray(Wa, np.float64)
    We = np.asarray(We, np.float64)
    Ws = np.asarray(Ws, np.float64)
    Wo = np.asarray(Wo, np.float64)
    wa2 = Wa[SELF:, 0]  # [28]
    A_self = We[:SELF] @ Wo[:H] + (Ws[:SELF] + Ws[SELF:]) @ Wo[H:]  # [36, 64]
    A_pool = We[SELF:] @ Wo[:H]  # [28, 64]
    c = (
        np.asarray(be, np.float64) @ Wo[:H]
        + np.asarray(bs, np.float64) @ Wo[H:]
        + np.asarray(bo, np.float64)
    )  # [64]

    WLp = np.zeros((128, NCH, J + 1), np.float32)
    Bp = np.zeros((J, NCH, 128), np.float32)
    FWp = np.zeros((128, NCH, H + 2), np.float32)
    for ch in range(NCH):
        for p in range(128):
            f = 128 * ch + p
            if f >= OBS:
                continue
            if f < SELF:
                Bp[:, ch, p] = 1.0  # ones block -> s for self features
                FWp[p, ch, 0:H] = A_self[f]
            else:
                j0, k = divmod(f - SELF, OTH)
                WLp[p, ch, j0] = wa2[k]
                Bp[j0, ch, p] = 1.0
                FWp[p, ch, 0:H] = A_pool[k]
    # s-row: pad feature 568 (p=56, ch=4) is 1.0 on the host; all-ones
    # selector row makes erep there = s; indicator FW column 64 routes it
    # into the final matmul output
    Bp[:, 4, 56] = 1.0
    FWp[56, 4, H] = 1.0
    return WLp, Bp, FWp, c.astype(np.float32)


def kernel(x, Wa, ba, We, be, Ws, bs, Wo, bo):
    import ml_dtypes

    from concourse import bass_utils

    x = np.asarray(x, np.float32)
    assert x.shape == (BATCH, OBS), x.shape
    # host-side bf16 cast + pad to 640 (XBAR alignment): device loads
    # contiguous bf16 directly, no on-chip conversion
    xpad = np.zeros((BATCH, PADF), dtype=ml_dtypes.bfloat16)
    xpad[:, :OBS] = x.astype(ml_dtypes.bfloat16)
    xpad[:, OBS] = 1.0  # s-row source: xt[56, c=4] = 1 so erep there yields s

    WLp, Bp, FWp, c = _fold_weights(Wa, ba, We, be, Ws, bs, Wo, bo)

    if "nc" not in _CACHE:
        _CACHE["nc"] = _build_nc()
    nc = _CACHE["nc"]

    in_maps = []
    for i in range(NCORES):
        in_maps.append(
            {
                "x_in": xpad[i * ROWS_PER_CORE : (i + 1) * ROWS_PER_CORE],
                "wl_in": WLp,
                "bsel_in": Bp,
                "fw_in": FWp,
            }
        )

    res = bass_utils.run_bass_kernel_spmd(
        nc,
        in_maps,
        core_ids=list(range(NCORES)),
        trace=_CACHE.get("trace", False),
        **_CACHE.get("run_kwargs", {}),
    )
    _CACHE["last_results"] = res

    out = np.concatenate([np.asarray(res.results[i]["out"]) for i in range(NCORES)], 0)
    if np.any(c):
        out = out + c[None, :]
    return out.astype(np.float32)
